# revision 3
# baseline (speedup 1.0000x reference)
"""Trainium2 Bass kernel for nn_Attention_12369505813001.

Computes, per batch b:
    qw    = query @ W_in.T                      [T, H]
    score = qw @ enc.T                          [T, S]
    p     = softmax(mask(score), axis=S)
    c     = p @ enc                             [T, H]
    out   = tanh(concat(query, c) @ W_out.T + b_out)

Shapes: B=32, T=512, S=1024, H=1024, fp32. Data-parallel over B across
8 NeuronCores (4 batches/core); no collectives.

Layout strategy (per core): keep the feature dim on partitions and T on
the free axis throughout ("transposed" layouts), so the PE contraction
dim always lands on partitions and no on-device transposes are needed:
    step1  qw^T[o,t]    = W_inT-tiles(stat) @ q^T(moving)     bf16x2, 3 MM/tile
    step2  score^T[s,t] = encT-tiles(stat)  @ qw^T(moving)    bf16x2
    softmax over s (partition+chunk axis): per-batch global max via
      free-axis reduce + GPSIMD partition all-reduce; exp on ACT with
      per-partition bias = -max + additive length mask; denominator via
      ones-vector matmul; normalization folded into c as a broadcast mul.
    step4  c~^T[h,t]    = enc-tiles(stat)   @ e^T(moving)     fp32r
    step5  out^T[o,t]   = tanh(WqT(stat) @ q^T + WcT(stat) @ cnorm + b)  fp32r

Precision: the softmax path (steps 1-2) uses two-term bf16 splits
(hi*hi + hi*lo + lo*hi accumulated in fp32 PSUM), measured ~4e-6 rel
matmul error; steps 4-5 use fp32r (~1.5e-4 rel, 1 cycle/row at N>=256).
Expected end-to-end absmax error vs the fp32 reference ~1e-3.

SBUF: big per-batch intermediates time-share two 16KB/partition slots
via pool tags (lifetimes are sequential in program order):
    big1: (qh|ql) -> score -> cnorm ; big2: (qwh|qwl) -> e
"""

from contextlib import ExitStack

import numpy as np
import ml_dtypes

import concourse.bass as bass
import concourse.bass_isa as bass_isa
import concourse.mybir as mybir
import concourse.tile as tile
from concourse import bacc
from concourse.bass_utils import run_bass_kernel_spmd

B, T, S, H = 32, 512, 1024, 1024
NCORES = 8
BPC = B // NCORES          # batches per core
HT = H // 128              # h/o chunk count
ST = S // 128              # s chunk count
P = 128

f32 = mybir.dt.float32
f32r = mybir.dt.float32r
bf16 = mybir.dt.bfloat16
AX = mybir.AxisListType.X
AF = mybir.ActivationFunctionType
SUB = mybir.AluOpType.subtract

MASKVAL = -1.0e38

_nc_cache = []

DEBUG = False
TRACE = False          # set by test.py to capture an NTFF/perfetto profile
LAST_RESULTS = None    # test.py reads exec_time_ns / trace path from here


def _build_nc():
    nc = bacc.Bacc("TRN2", target_bir_lowering=False, debug=False)

    qT = nc.dram_tensor("qT", [BPC, H, T], f32, kind="ExternalInput")
    eT = nc.dram_tensor("eT", [BPC, H, 2, S], bf16, kind="ExternalInput")
    enc = nc.dram_tensor("enc", [BPC, S, H], f32, kind="ExternalInput")
    maskc = nc.dram_tensor("maskc", [BPC, P, ST], f32, kind="ExternalInput")
    Wih = nc.dram_tensor("Wih", [H, H], bf16, kind="ExternalInput")
    Wil = nc.dram_tensor("Wil", [H, H], bf16, kind="ExternalInput")
    Wq = nc.dram_tensor("Wq", [H, H], f32, kind="ExternalInput")
    Wc = nc.dram_tensor("Wc", [H, H], f32, kind="ExternalInput")
    bo = nc.dram_tensor("bo", [P, HT], f32, kind="ExternalInput")
    onesv = nc.dram_tensor("onesv", [P, 1], f32, kind="ExternalInput")
    outT = nc.dram_tensor("outT", [BPC, H, T], f32, kind="ExternalOutput")
    if DEBUG:
        score_d = nc.dram_tensor("score_d", [BPC, S, T], f32, kind="ExternalOutput")
        e_d = nc.dram_tensor("e_d", [BPC, S, T], f32, kind="ExternalOutput")
        rden_d = nc.dram_tensor("rden_d", [BPC, 1, T], f32, kind="ExternalOutput")
        cn_d = nc.dram_tensor("cn_d", [BPC, H, T], f32, kind="ExternalOutput")
        qw_d = nc.dram_tensor("qw_d", [BPC, 2, H, T], bf16, kind="ExternalOutput")
        qwps_d = nc.dram_tensor("qwps_d", [BPC, H, T], f32, kind="ExternalOutput")
        qs_d = nc.dram_tensor("qs_d", [BPC, 2, H, T], bf16, kind="ExternalOutput")

    with tile.TileContext(nc) as tc, ExitStack() as ctx:
        wp = ctx.enter_context(tc.tile_pool(name="wp", bufs=1))
        pb = ctx.enter_context(tc.tile_pool(name="pb", bufs=1))
        pq = ctx.enter_context(tc.tile_pool(name="pq", bufs=2))
        pq32 = ctx.enter_context(tc.tile_pool(name="pq32", bufs=1))
        sp = ctx.enter_context(tc.tile_pool(name="sp", bufs=2))
        etp = ctx.enter_context(tc.tile_pool(name="etp", bufs=8))
        enp = ctx.enter_context(tc.tile_pool(name="enp", bufs=8))
        otp = ctx.enter_context(tc.tile_pool(name="otp", bufs=2))
        psA = ctx.enter_context(tc.tile_pool(name="psA", bufs=2, space="PSUM"))
        psB = ctx.enter_context(tc.tile_pool(name="psB", bufs=2, space="PSUM"))
        psC = ctx.enter_context(tc.tile_pool(name="psC", bufs=2, space="PSUM"))
        psD = ctx.enter_context(tc.tile_pool(name="psD", bufs=1, space="PSUM"))

        # --- persistent weights ---
        wih = wp.tile([P, HT, H], bf16)
        nc.sync.dma_start(out=wih, in_=Wih[:, :].rearrange("(k p) o -> p k o", p=P))
        wil = wp.tile([P, HT, H], bf16)
        nc.sync.dma_start(out=wil, in_=Wil[:, :].rearrange("(k p) o -> p k o", p=P))
        wq = wp.tile([P, HT, H], f32r)
        nc.sync.dma_start(
            out=wq, in_=Wq[:, :].rearrange("(k p) o -> p k o", p=P).bitcast(f32r))
        wc = wp.tile([P, HT, H], f32r)
        nc.sync.dma_start(
            out=wc, in_=Wc[:, :].rearrange("(k p) o -> p k o", p=P).bitcast(f32r))
        bo_sb = wp.tile([P, HT], f32)
        nc.sync.dma_start(out=bo_sb, in_=bo[:, :])
        mask_sb = wp.tile([P, BPC, ST], f32)
        nc.sync.dma_start(out=mask_sb, in_=maskc[:, :, :].rearrange("b p m -> p b m"))
        ones_sb = wp.tile([P, 1], f32r)
        nc.sync.dma_start(out=ones_sb, in_=onesv[:, :].bitcast(f32r))

        for b in range(BPC):
            # --- load q^T, split into bf16 hi/lo ---
            # q is loaded twice: once as plain f32 for the bf16 hi/lo split
            # (any f32r-typed write or read path applies RN-11 rounding, which
            # would silently cap the bf16x2 softmax path at fp32r precision),
            # and once f32r-typed for the step-5 fp32r matmul.
            q_r32 = pq32.tile([P, HT, T], f32, tag="q_r32")
            nc.sync.dma_start(
                out=q_r32,
                in_=qT[b, :, :].rearrange("(k p) t -> p k t", p=P))
            q_r = pq.tile([P, HT, T], f32r, tag="q_r")
            nc.sync.dma_start(
                out=q_r,
                in_=qT[b, :, :].rearrange("(k p) t -> p k t", p=P).bitcast(f32r))
            qhl = pb.tile([P, 2, HT, T], bf16, tag="big1")
            for k in range(HT):
                nc.scalar.copy(qhl[:, 0, k, :], q_r32[:, k, :])
                nc.vector.tensor_sub(qhl[:, 1, k, :], q_r32[:, k, :],
                                     qhl[:, 0, k, :])
                if DEBUG:
                    nc.sync.dma_start(out=qs_d[b, 0, 128 * k:128 * (k + 1), :],
                                      in_=qhl[:, 0, k, :])
                    nc.sync.dma_start(out=qs_d[b, 1, 128 * k:128 * (k + 1), :],
                                      in_=qhl[:, 1, k, :])

            # --- step 1: qw^T = W_inT @ q^T  (bf16x2) ---
            qwhl = pb.tile([P, 2, HT, T], bf16, tag="big2")
            for m in range(HT):
                qw_ps = psA.tile([P, T], f32, tag="qo", name=f"qw_{b}_{m}")
                i = 0
                msl = slice(128 * m, 128 * (m + 1))
                for k in range(HT):
                    for X, Y in ((wih, 0), (wih, 1), (wil, 0)):
                        nc.tensor.matmul(qw_ps, X[:, k, msl], qhl[:, Y, k, :],
                                         start=(i == 0), stop=(i == 3 * HT - 1))
                        i += 1
                nc.scalar.copy(qwhl[:, 0, m, :], qw_ps)
                nc.vector.tensor_sub(qwhl[:, 1, m, :], qw_ps, qwhl[:, 0, m, :])
                if DEBUG:
                    qwf = otp.tile([P, T], f32, tag="qwf")
                    nc.vector.tensor_copy(qwf, qw_ps)
                    nc.sync.dma_start(out=qwps_d[b, 128 * m:128 * (m + 1), :],
                                      in_=qwf)
                    nc.sync.dma_start(
                        out=qw_d[b, 0, 128 * m:128 * (m + 1), :],
                        in_=qwhl[:, 0, m, :])
                    nc.sync.dma_start(
                        out=qw_d[b, 1, 128 * m:128 * (m + 1), :],
                        in_=qwhl[:, 1, m, :])

            # --- step 2: score^T = encT @ qw^T  (bf16x2) + partial max tree ---
            score = pb.tile([P, ST, T], f32, tag="big1")
            smax = sp.tile([P, T], f32, tag="smax")
            for m in range(ST):
                sc_ps = psB.tile([P, T], f32, tag="sc", name=f"sc_{b}_{m}")
                i = 0
                for k in range(HT):
                    et = etp.tile([P, 2, 128], bf16, tag="et")
                    nc.sync.dma_start(
                        out=et,
                        in_=eT[b, 128 * k:128 * (k + 1), :,
                               128 * m:128 * (m + 1)])
                    for X, Y in ((0, 0), (0, 1), (1, 0)):
                        nc.tensor.matmul(sc_ps, et[:, X, :], qwhl[:, Y, k, :],
                                         start=(i == 0), stop=(i == 3 * HT - 1))
                        i += 1
                nc.scalar.copy(score[:, m, :], sc_ps)
                if m == 0:
                    nc.vector.tensor_copy(smax, sc_ps)
                else:
                    nc.vector.tensor_max(smax, smax, sc_ps)

            # --- per-t max over all s: all-reduce across partitions ---
            # (max over masked-but-real score rows is included; that only
            # shifts the offset upward by a few units, which cancels in the
            # softmax ratio and cannot underflow the per-column top.)
            smax_all = sp.tile([P, T], f32, tag="smax_all")
            nc.gpsimd.partition_all_reduce(smax_all, smax, channels=P,
                                           reduce_op=bass_isa.ReduceOp.max)

            # --- exp (score - m_t -> e, fp32r) + denominator ---
            e = pb.tile([P, ST, T], f32r, tag="big2")
            for m in range(ST):
                nc.vector.tensor_sub(score[:, m, :], score[:, m, :], smax_all)
                nc.scalar.activation(e[:, m, :], score[:, m, :], AF.Exp,
                                     bias=mask_sb[:, b, m:m + 1])
                if DEBUG:
                    nc.sync.dma_start(
                        out=score_d[b, 128 * m:128 * (m + 1), :],
                        in_=score[:, m, :])
                    nc.sync.dma_start(
                        out=e_d[b, 128 * m:128 * (m + 1), :].bitcast(f32r),
                        in_=e[:, m, :])
            den_ps = psD.tile([1, T], f32, tag="den", name=f"den_{b}")
            for m in range(ST):
                nc.tensor.matmul(den_ps, ones_sb, e[:, m, :],
                                 start=(m == 0), stop=(m == ST - 1))
            rden = sp.tile([1, T], f32, tag="rden")
            nc.vector.reciprocal(rden, den_ps)
            rdenb = sp.tile([P, T], f32, tag="rdenb")
            nc.gpsimd.partition_broadcast(rdenb, rden, channels=P)
            if DEBUG:
                nc.sync.dma_start(out=rden_d[b, :, :], in_=rden)

            # --- step 4: c~^T = enc @ e^T (fp32r), normalize by 1/denom ---
            cn = pb.tile([P, HT, T], f32r, tag="big1")
            for m in range(HT):
                c_ps = psC.tile([P, T], f32, tag="c", name=f"c_{b}_{m}")
                for k in range(ST):
                    en = enp.tile([P, 128], f32r, tag="en")
                    nc.sync.dma_start(
                        out=en,
                        in_=enc[b, 128 * k:128 * (k + 1),
                                128 * m:128 * (m + 1)].bitcast(f32r))
                    nc.tensor.matmul(c_ps, en, e[:, k, :],
                                     start=(k == 0), stop=(k == ST - 1))
                nc.vector.tensor_mul(cn[:, m, :], c_ps, rdenb)
                if DEBUG:
                    nc.sync.dma_start(
                        out=cn_d[b, 128 * m:128 * (m + 1), :].bitcast(f32r),
                        in_=cn[:, m, :])

            # --- step 5: out^T = tanh(WqT @ q^T + WcT @ cnorm + b) ---
            for m in range(HT):
                o_ps = psA.tile([P, T], f32, tag="qo", name=f"o_{b}_{m}")
                msl = slice(128 * m, 128 * (m + 1))
                for k in range(HT):
                    nc.tensor.matmul(o_ps, wq[:, k, msl], q_r[:, k, :],
                                     start=(k == 0), stop=False)
                for k in range(HT):
                    nc.tensor.matmul(o_ps, wc[:, k, msl], cn[:, k, :],
                                     start=False, stop=(k == HT - 1))
                ot = otp.tile([P, T], f32, tag="ot")
                nc.scalar.activation(ot, o_ps, AF.Tanh, bias=bo_sb[:, m:m + 1])
                nc.sync.dma_start(out=outT[b, 128 * m:128 * (m + 1), :], in_=ot)

    nc.compile()
    return nc


def _bf16_split(x):
    hi = x.astype(ml_dtypes.bfloat16)
    lo = (x - hi.astype(np.float32)).astype(ml_dtypes.bfloat16)
    return hi, lo


def kernel(query, encoder_outputs, src_lengths, W_in, W_out, b_out):
    query = np.asarray(query, dtype=np.float32)
    encoder_outputs = np.ascontiguousarray(np.asarray(encoder_outputs, np.float32))
    src_lengths = np.asarray(src_lengths)
    W_in = np.asarray(W_in, dtype=np.float32)
    W_out = np.asarray(W_out, dtype=np.float32)
    b_out = np.asarray(b_out, dtype=np.float32)

    # --- shared (weight) inputs ---
    W_inT = np.ascontiguousarray(W_in.T)                    # [h, o]
    Wih, Wil = _bf16_split(W_inT)
    Wq = np.ascontiguousarray(W_out[:, :H].T)               # [h, o]
    Wc = np.ascontiguousarray(W_out[:, H:].T)               # [h, o]
    bo = np.ascontiguousarray(b_out.reshape(HT, P).T)       # [p, m]
    onesv = np.ones((P, 1), dtype=np.float32)

    # --- per-core shards ---
    in_maps = []
    for c in range(NCORES):
        bs = slice(c * BPC, (c + 1) * BPC)
        q = query[bs]                                       # [BPC, T, H]
        encs = encoder_outputs[bs]                          # [BPC, S, H]
        lens = np.asarray(src_lengths[bs], dtype=np.int64)

        qTa = np.ascontiguousarray(q.transpose(0, 2, 1))    # [BPC, H, T]
        encTa = np.ascontiguousarray(encs.transpose(0, 2, 1))  # [BPC, H, S]
        eh, el = _bf16_split(encTa)
        eTa = np.ascontiguousarray(np.stack([eh, el], axis=2))  # [BPC, H, 2, S]

        maskca = np.zeros((BPC, P, ST), dtype=np.float32)
        pos = (np.arange(ST)[None, :] * P + np.arange(P)[:, None])  # [P, ST]
        for j in range(BPC):
            maskca[j][pos >= lens[j]] = MASKVAL

        in_maps.append({
            "qT": qTa, "eT": eTa, "enc": encs, "maskc": maskca,
            "Wih": Wih, "Wil": Wil, "Wq": Wq, "Wc": Wc,
            "bo": bo, "onesv": onesv,
        })

    if not _nc_cache:
        _nc_cache.append(_build_nc())
    nc = _nc_cache[0]

    res = run_bass_kernel_spmd(nc, in_maps, core_ids=list(range(NCORES)),
                               trace=TRACE)
    global LAST_RESULTS
    LAST_RESULTS = res

    out = np.empty((B, T, H), dtype=np.float32)
    for c in range(NCORES):
        o = res.results[c]["outT"]                          # [BPC, H, T]
        out[c * BPC:(c + 1) * BPC] = o.transpose(0, 2, 1)
    return out



# revision 4
# speedup vs baseline: 1.0380x; 1.0380x over previous
"""Trainium2 Bass kernel for nn_Attention_12369505813001.

Computes, per batch b:
    qw    = query @ W_in.T                      [T, H]
    score = qw @ enc.T                          [T, S]
    p     = softmax(mask(score), axis=S)
    c     = p @ enc                             [T, H]
    out   = tanh(concat(query, c) @ W_out.T + b_out)

Shapes: B=32, T=512, S=1024, H=1024, fp32. Data-parallel over B across
8 NeuronCores (4 batches/core); no collectives.

Layout strategy (per core): keep the feature dim on partitions and T on
the free axis throughout ("transposed" layouts), so the PE contraction
dim always lands on partitions and no on-device transposes are needed:
    step1  qw^T[o,t]    = W_inT-tiles(stat) @ q^T(moving)     bf16x2, 3 MM/tile
    step2  score^T[s,t] = encT-tiles(stat)  @ qw^T(moving)    bf16x2
    softmax over s (partition+chunk axis): per-batch global max via
      free-axis reduce + GPSIMD partition all-reduce; exp on ACT with
      per-partition bias = -max + additive length mask; denominator via
      ones-vector matmul; normalization folded into c as a broadcast mul.
    step4  c~^T[h,t]    = enc-tiles(stat)   @ e^T(moving)     bf16
    step5  out^T[o,t]   = tanh(WqT(stat) @ q^T + WcT(stat) @ cnorm + b)  fp16

Precision: the softmax path (steps 1-2) uses two-term bf16 splits
(hi*hi + hi*lo + lo*hi accumulated in fp32 PSUM) — the softmax
amplifies score errors, so single-pass dtypes (bf16/f32r/fp16) all
fail the 2e-2 gate there. The context (step 4) runs in plain bf16 with
e stored bf16 (denominator computed from the same quantized e, so the
softmax simplex property is preserved), and step 5 runs in fp16
(11-bit mantissa at full PE rate, FWL-eligible unlike fp32r).
CPU-simulated end-to-end rel err of this mix: ~8.3e-3 (gate 2e-2).

All matmuls run at the 1-row/cycle warm rate (~216 ns at N=512).
The q^T hi/lo bf16 split and the fp16 copy of q^T are computed on the
host and DMA'd, removing 16 ACT/DVE ops per batch.

SBUF: per-batch intermediates time-share slots via pool tags, with
bufs=2 so consecutive batches pipeline (batch b+1's step-1 matmuls
overlap batch b's softmax/context/output phases):
    big1: score -> cn16 ; big2: qwhl -> e ; pq: qhl + q16
"""

from contextlib import ExitStack

import numpy as np
import ml_dtypes

import concourse.bass as bass
import concourse.bass_isa as bass_isa
import concourse.mybir as mybir
import concourse.tile as tile
from concourse import bacc
from concourse.bass_utils import run_bass_kernel_spmd

B, T, S, H = 32, 512, 1024, 1024
NCORES = 8
BPC = B // NCORES          # batches per core
HT = H // 128              # h/o chunk count
ST = S // 128              # s chunk count
P = 128

f32 = mybir.dt.float32
bf16 = mybir.dt.bfloat16
f16 = mybir.dt.float16
AX = mybir.AxisListType.X
AF = mybir.ActivationFunctionType

MASKVAL = -1.0e38

_nc_cache = []

TRACE = False          # set by test.py to capture an NTFF/perfetto profile
LAST_RESULTS = None    # test.py reads exec_time_ns / trace path from here


def _build_nc():
    nc = bacc.Bacc("TRN2", target_bir_lowering=False, debug=False)

    qhl_d = nc.dram_tensor("qhl", [BPC, H, 2, T], bf16, kind="ExternalInput")
    q16_d = nc.dram_tensor("q16", [BPC, H, T], f16, kind="ExternalInput")
    eT = nc.dram_tensor("eT", [BPC, H, 2, S], bf16, kind="ExternalInput")
    encb = nc.dram_tensor("encb", [BPC, S, H], bf16, kind="ExternalInput")
    maskc = nc.dram_tensor("maskc", [BPC, P, ST], f32, kind="ExternalInput")
    Wih = nc.dram_tensor("Wih", [H, H], bf16, kind="ExternalInput")
    Wil = nc.dram_tensor("Wil", [H, H], bf16, kind="ExternalInput")
    W16 = nc.dram_tensor("W16", [H, 2, H], f16, kind="ExternalInput")
    bo = nc.dram_tensor("bo", [P, HT], f32, kind="ExternalInput")
    onesv = nc.dram_tensor("onesv", [P, 1], bf16, kind="ExternalInput")
    outT = nc.dram_tensor("outT", [BPC, H, T], f32, kind="ExternalOutput")

    with tile.TileContext(nc) as tc, ExitStack() as ctx:
        wp = ctx.enter_context(tc.tile_pool(name="wp", bufs=1))
        pq = ctx.enter_context(tc.tile_pool(name="pq", bufs=2))
        pb = ctx.enter_context(tc.tile_pool(name="pb", bufs=2))
        sp = ctx.enter_context(tc.tile_pool(name="sp", bufs=2))
        etp = ctx.enter_context(tc.tile_pool(name="etp", bufs=8))
        enp = ctx.enter_context(tc.tile_pool(name="enp", bufs=16))
        otp = ctx.enter_context(tc.tile_pool(name="otp", bufs=2))
        psA = ctx.enter_context(tc.tile_pool(name="psA", bufs=2, space="PSUM"))
        psB = ctx.enter_context(tc.tile_pool(name="psB", bufs=2, space="PSUM"))
        psC = ctx.enter_context(tc.tile_pool(name="psC", bufs=2, space="PSUM"))
        psD = ctx.enter_context(tc.tile_pool(name="psD", bufs=2, space="PSUM"))

        # --- persistent weights ---
        wih = wp.tile([P, HT, H], bf16)
        nc.sync.dma_start(out=wih, in_=Wih[:, :].rearrange("(k p) o -> p k o", p=P))
        wil = wp.tile([P, HT, H], bf16)
        nc.sync.dma_start(out=wil, in_=Wil[:, :].rearrange("(k p) o -> p k o", p=P))
        w16 = wp.tile([P, 2, HT, H], f16)
        nc.sync.dma_start(
            out=w16, in_=W16[:, :, :].rearrange("(k p) c o -> p c k o", p=P))
        bo_sb = wp.tile([P, HT], f32)
        nc.sync.dma_start(out=bo_sb, in_=bo[:, :])
        mask_sb = wp.tile([P, BPC, ST], f32)
        nc.sync.dma_start(out=mask_sb, in_=maskc[:, :, :].rearrange("b p m -> p b m"))
        ones_sb = wp.tile([P, 1], bf16)
        nc.sync.dma_start(out=ones_sb, in_=onesv[:, :])

        for b in range(BPC):
            # --- load q^T hi/lo (host-split) and fp16 q^T ---
            qhl = pq.tile([P, 2, HT, T], bf16, tag="qhl")
            nc.sync.dma_start(
                out=qhl,
                in_=qhl_d[b, :, :, :].rearrange("(k p) c t -> p c k t", p=P))
            q16 = pq.tile([P, HT, T], f16, tag="q16")
            nc.sync.dma_start(
                out=q16,
                in_=q16_d[b, :, :].rearrange("(k p) t -> p k t", p=P))

            # --- step 1: qw^T = W_inT @ q^T  (bf16x2) ---
            qwhl = pb.tile([P, 2, HT, T], bf16, tag="big2")
            for m in range(HT):
                qw_ps = psA.tile([P, T], f32, tag="qo", name=f"qw_{b}_{m}")
                i = 0
                msl = slice(128 * m, 128 * (m + 1))
                for k in range(HT):
                    for X, Y in ((wih, 0), (wih, 1), (wil, 0)):
                        nc.tensor.matmul(qw_ps, X[:, k, msl], qhl[:, Y, k, :],
                                         start=(i == 0), stop=(i == 3 * HT - 1))
                        i += 1
                nc.scalar.copy(qwhl[:, 0, m, :], qw_ps)
                nc.vector.tensor_sub(qwhl[:, 1, m, :], qw_ps, qwhl[:, 0, m, :])

            # --- step 2: score^T = encT @ qw^T  (bf16x2) + partial max tree ---
            score = pb.tile([P, ST, T], f32, tag="big1")
            smax = sp.tile([P, T], f32, tag="smax")
            for m in range(ST):
                sc_ps = psB.tile([P, T], f32, tag="sc", name=f"sc_{b}_{m}")
                i = 0
                for k in range(HT):
                    et = etp.tile([P, 2, 128], bf16, tag="et")
                    nc.sync.dma_start(
                        out=et,
                        in_=eT[b, 128 * k:128 * (k + 1), :,
                               128 * m:128 * (m + 1)])
                    for X, Y in ((0, 0), (0, 1), (1, 0)):
                        nc.tensor.matmul(sc_ps, et[:, X, :], qwhl[:, Y, k, :],
                                         start=(i == 0), stop=(i == 3 * HT - 1))
                        i += 1
                nc.scalar.copy(score[:, m, :], sc_ps)
                if m == 0:
                    nc.vector.tensor_copy(smax, sc_ps)
                else:
                    nc.vector.tensor_max(smax, smax, sc_ps)

            # --- per-t max over all s: all-reduce across partitions ---
            # (max over masked-but-real score rows is included; that only
            # shifts the offset upward by a few units, which cancels in the
            # softmax ratio and cannot underflow the per-column top.)
            smax_all = sp.tile([P, T], f32, tag="smax_all")
            nc.gpsimd.partition_all_reduce(smax_all, smax, channels=P,
                                           reduce_op=bass_isa.ReduceOp.max)

            # --- exp (score - m_t -> e, bf16) + denominator ---
            e = pb.tile([P, ST, T], bf16, tag="big2")
            for m in range(ST):
                nc.vector.tensor_sub(score[:, m, :], score[:, m, :], smax_all)
                nc.scalar.activation(e[:, m, :], score[:, m, :], AF.Exp,
                                     bias=mask_sb[:, b, m:m + 1])
            den_ps = psD.tile([1, T], f32, tag="den", name=f"den_{b}")
            for m in range(ST):
                nc.tensor.matmul(den_ps, ones_sb, e[:, m, :],
                                 start=(m == 0), stop=(m == ST - 1))
            rden = sp.tile([1, T], f32, tag="rden")
            nc.vector.reciprocal(rden, den_ps)
            rdenb = sp.tile([P, T], f32, tag="rdenb")
            nc.gpsimd.partition_broadcast(rdenb, rden, channels=P)

            # --- step 4: c~^T = enc @ e^T (bf16), normalize by 1/denom ---
            cn = pb.tile([P, HT, T], f16, tag="big1")
            for m in range(HT):
                c_ps = psC.tile([P, T], f32, tag="c", name=f"c_{b}_{m}")
                for k in range(ST):
                    en = enp.tile([P, 128], bf16, tag="en")
                    nc.sync.dma_start(
                        out=en,
                        in_=encb[b, 128 * k:128 * (k + 1),
                                 128 * m:128 * (m + 1)])
                    nc.tensor.matmul(c_ps, en, e[:, k, :],
                                     start=(k == 0), stop=(k == ST - 1))
                nc.vector.tensor_mul(cn[:, m, :], c_ps, rdenb)

            # --- step 5: out^T = tanh(WqT @ q^T + WcT @ cnorm + b) ---
            for m in range(HT):
                o_ps = psA.tile([P, T], f32, tag="qo", name=f"o_{b}_{m}")
                msl = slice(128 * m, 128 * (m + 1))
                for k in range(HT):
                    nc.tensor.matmul(o_ps, w16[:, 0, k, msl], q16[:, k, :],
                                     start=(k == 0), stop=False)
                for k in range(HT):
                    nc.tensor.matmul(o_ps, w16[:, 1, k, msl], cn[:, k, :],
                                     start=False, stop=(k == HT - 1))
                ot = otp.tile([P, T], f32, tag="ot")
                nc.scalar.activation(ot, o_ps, AF.Tanh, bias=bo_sb[:, m:m + 1])
                nc.sync.dma_start(out=outT[b, 128 * m:128 * (m + 1), :], in_=ot)

    nc.compile()
    return nc


def _bf16_split(x):
    hi = x.astype(ml_dtypes.bfloat16)
    lo = (x - hi.astype(np.float32)).astype(ml_dtypes.bfloat16)
    return hi, lo


def kernel(query, encoder_outputs, src_lengths, W_in, W_out, b_out):
    query = np.asarray(query, dtype=np.float32)
    encoder_outputs = np.ascontiguousarray(np.asarray(encoder_outputs, np.float32))
    src_lengths = np.asarray(src_lengths)
    W_in = np.asarray(W_in, dtype=np.float32)
    W_out = np.asarray(W_out, dtype=np.float32)
    b_out = np.asarray(b_out, dtype=np.float32)

    # --- shared (weight) inputs ---
    W_inT = np.ascontiguousarray(W_in.T)                    # [h, o]
    Wih, Wil = _bf16_split(W_inT)
    W16 = np.ascontiguousarray(
        np.stack([W_out[:, :H].T, W_out[:, H:].T], axis=1)  # [h, 2, o]
    ).astype(np.float16)
    bo = np.ascontiguousarray(b_out.reshape(HT, P).T)       # [p, m]
    onesv = np.ones((P, 1), dtype=ml_dtypes.bfloat16)

    # --- per-core shards ---
    in_maps = []
    for c in range(NCORES):
        bs = slice(c * BPC, (c + 1) * BPC)
        q = query[bs]                                       # [BPC, T, H]
        encs = encoder_outputs[bs]                          # [BPC, S, H]
        lens = np.asarray(src_lengths[bs], dtype=np.int64)

        qTa = np.ascontiguousarray(q.transpose(0, 2, 1))    # [BPC, H, T]
        qh, ql = _bf16_split(qTa)
        qhla = np.ascontiguousarray(np.stack([qh, ql], axis=2))  # [BPC, H, 2, T]
        q16a = qTa.astype(np.float16)                       # [BPC, H, T]
        encTa = np.ascontiguousarray(encs.transpose(0, 2, 1))  # [BPC, H, S]
        eh, el = _bf16_split(encTa)
        eTa = np.ascontiguousarray(np.stack([eh, el], axis=2))  # [BPC, H, 2, S]
        encba = encs.astype(ml_dtypes.bfloat16)             # [BPC, S, H]

        maskca = np.zeros((BPC, P, ST), dtype=np.float32)
        pos = (np.arange(ST)[None, :] * P + np.arange(P)[:, None])  # [P, ST]
        for j in range(BPC):
            maskca[j][pos >= lens[j]] = MASKVAL

        in_maps.append({
            "qhl": qhla, "q16": q16a, "eT": eTa, "encb": encba,
            "maskc": maskca, "Wih": Wih, "Wil": Wil, "W16": W16,
            "bo": bo, "onesv": onesv,
        })

    if not _nc_cache:
        _nc_cache.append(_build_nc())
    nc = _nc_cache[0]

    res = run_bass_kernel_spmd(nc, in_maps, core_ids=list(range(NCORES)),
                               trace=TRACE)
    global LAST_RESULTS
    LAST_RESULTS = res

    out = np.empty((B, T, H), dtype=np.float32)
    for c in range(NCORES):
        o = res.results[c]["outT"]                          # [BPC, H, T]
        out[c * BPC:(c + 1) * BPC] = o.transpose(0, 2, 1)
    return out


# revision 5
# speedup vs baseline: 1.1806x; 1.1373x over previous
"""Trainium2 Bass kernel for nn_Attention_12369505813001.

Computes, per batch b:
    qw    = query @ W_in.T                      [T, H]
    score = qw @ enc.T                          [T, S]
    p     = softmax(mask(score), axis=S)
    c     = p @ enc                             [T, H]
    out   = tanh(concat(query, c) @ W_out.T + b_out)

Shapes: B=32, T=512, S=1024, H=1024, fp32. Data-parallel over B across
8 NeuronCores (4 batches/core); no collectives.

Layout strategy (per core): keep the feature dim on partitions and T on
the free axis throughout ("transposed" layouts), so the PE contraction
dim always lands on partitions and no on-device transposes are needed:
    step1  qw^T[o,t]    = W_inT-tiles(stat) @ q^T(moving)     bf16x2, 3 MM/tile
    step2  score^T[s,t] = encT-tiles(stat)  @ qw^T(moving)    bf16x2
    softmax over s (partition+chunk axis): per-batch per-t max via
      free-axis reduce + GPSIMD partition all-reduce; exp on ACT with
      per-partition bias = additive length mask; denominator via an
      all-ones [128x128] stationary matmul (gives den broadcast to all
      128 partitions directly -> full-lane DVE reciprocal).
    step4  c~^T[h,t]    = enc-tiles(stat)   @ e^T(moving)     bf16
    step5  out^T[o,t]   = tanh(WqT(stat) @ q^T + WcT(stat) @ cnorm + b)  fp16

Precision: the softmax path (steps 1-2) uses two-term bf16 splits
(hi*hi + hi*lo + lo*hi accumulated in fp32 PSUM) — the softmax
amplifies score errors, so single-pass dtypes (bf16/f32r/fp16) all
fail the 2e-2 gate there. The context (step 4) runs in plain bf16 with
e stored bf16 (denominator computed from the same quantized e, so the
softmax simplex property is preserved), and step 5 runs in fp16
(11-bit mantissa at full PE rate, FWL-eligible unlike fp32r).
CPU-simulated end-to-end rel err of this mix: ~8.3e-3 (gate 2e-2).

Scheduling: the per-engine instruction streams are ordered at compile
time, so the batch loop is software-pipelined by hand into
A(load+step1+step2) / B(softmax+den) / C(context+step5) phases emitted
as A0 A1 B0 C0 B1 A2 C1 B2 A3 C2 B3 C3 — every B's vector/scalar
softmax work has another batch's matmuls in flight on the PE.
SBUF big tiles rotate through bufs=2 pool tags
(big1: score->cn16, big2: qwhl->e); this emission order keeps every
tag's producer after the previous consumer. DMAs are spread across the
two HWDGE queues (sync: weights/encT/enc; scalar: q-side + stores).
"""

from contextlib import ExitStack

import numpy as np
import ml_dtypes

import concourse.bass as bass
import concourse.bass_isa as bass_isa
import concourse.mybir as mybir
import concourse.tile as tile
from concourse import bacc
from concourse.bass_utils import run_bass_kernel_spmd

B, T, S, H = 32, 512, 1024, 1024
NCORES = 8
BPC = B // NCORES          # batches per core
HT = H // 128              # h/o chunk count
ST = S // 128              # s chunk count
P = 128

f32 = mybir.dt.float32
bf16 = mybir.dt.bfloat16
f16 = mybir.dt.float16
AX = mybir.AxisListType.X
AF = mybir.ActivationFunctionType

MASKVAL = -1.0e38

_nc_cache = []

TRACE = False          # set by test.py to capture an NTFF/perfetto profile
LAST_RESULTS = None    # test.py reads exec_time_ns / trace path from here


def _build_nc():
    nc = bacc.Bacc("TRN2", target_bir_lowering=False, debug=False)

    qhl_d = nc.dram_tensor("qhl", [BPC, H, 2, T], bf16, kind="ExternalInput")
    q16_d = nc.dram_tensor("q16", [BPC, H, T], f16, kind="ExternalInput")
    eT = nc.dram_tensor("eT", [BPC, H, 2, S], bf16, kind="ExternalInput")
    encb = nc.dram_tensor("encb", [BPC, S, H], bf16, kind="ExternalInput")
    maskc = nc.dram_tensor("maskc", [BPC, P, ST], f32, kind="ExternalInput")
    Wih = nc.dram_tensor("Wih", [H, H], bf16, kind="ExternalInput")
    Wil = nc.dram_tensor("Wil", [H, H], bf16, kind="ExternalInput")
    W16 = nc.dram_tensor("W16", [H, 2, H], f16, kind="ExternalInput")
    bo = nc.dram_tensor("bo", [P, HT], f32, kind="ExternalInput")
    onesv = nc.dram_tensor("onesv", [P, P], bf16, kind="ExternalInput")
    outT = nc.dram_tensor("outT", [BPC, H, T], f32, kind="ExternalOutput")

    with tile.TileContext(nc) as tc, ExitStack() as ctx:
        wp = ctx.enter_context(tc.tile_pool(name="wp", bufs=1))
        pq = ctx.enter_context(tc.tile_pool(name="pq", bufs=2))
        pq16 = ctx.enter_context(tc.tile_pool(name="pq16", bufs=1))
        pb = ctx.enter_context(tc.tile_pool(name="pb", bufs=2))
        sp = ctx.enter_context(tc.tile_pool(name="sp", bufs=2))
        etp = ctx.enter_context(tc.tile_pool(name="etp", bufs=6))
        enc_p = ctx.enter_context(tc.tile_pool(name="enc_p", bufs=1))
        otp = ctx.enter_context(tc.tile_pool(name="otp", bufs=2))
        psQ = ctx.enter_context(tc.tile_pool(name="psQ", bufs=2, space="PSUM"))
        psO = ctx.enter_context(tc.tile_pool(name="psO", bufs=2, space="PSUM"))
        psC = ctx.enter_context(tc.tile_pool(name="psC", bufs=3, space="PSUM"))
        psD = ctx.enter_context(tc.tile_pool(name="psD", bufs=1, space="PSUM"))

        # --- persistent weights (w16 is emitted later: first use is C0) ---
        wih = wp.tile([P, HT, H], bf16)
        nc.sync.dma_start(out=wih, in_=Wih[:, :].rearrange("(k p) o -> p k o", p=P))
        wil = wp.tile([P, HT, H], bf16)
        nc.sync.dma_start(out=wil, in_=Wil[:, :].rearrange("(k p) o -> p k o", p=P))
        bo_sb = wp.tile([P, HT], f32)
        nc.sync.dma_start(out=bo_sb, in_=bo[:, :])
        mask_sb = wp.tile([P, BPC, ST], f32)
        nc.sync.dma_start(out=mask_sb, in_=maskc[:, :, :].rearrange("b p m -> p b m"))
        ones_sb = wp.tile([P, P], bf16)
        nc.sync.dma_start(out=ones_sb, in_=onesv[:, :])
        w16 = wp.tile([P, 2, HT, H], f16)

        st = {}

        def phase_A(b):
            # --- load q^T hi/lo (host-split); step 1; step 2 + max tree ---
            qhl = pq.tile([P, 2, HT, T], bf16, tag="qhl")
            nc.scalar.dma_start(
                out=qhl,
                in_=qhl_d[b, :, :, :].rearrange("(k p) c t -> p c k t", p=P))

            qwhl = pb.tile([P, 2, HT, T], bf16, tag="big2")
            for m in range(HT):
                qw_ps = psQ.tile([P, T], f32, tag="qs", name=f"qw_{b}_{m}")
                i = 0
                msl = slice(128 * m, 128 * (m + 1))
                for k in range(HT):
                    for X, Y in ((wih, 0), (wih, 1), (wil, 0)):
                        nc.tensor.matmul(qw_ps, X[:, k, msl], qhl[:, Y, k, :],
                                         start=(i == 0), stop=(i == 3 * HT - 1))
                        i += 1
                nc.scalar.copy(qwhl[:, 0, m, :], qw_ps)
                nc.vector.tensor_sub(qwhl[:, 1, m, :], qw_ps, qwhl[:, 0, m, :])

            score = pb.tile([P, ST, T], f32, tag="big1")
            smax = sp.tile([P, T], f32, tag="smax")
            for m in range(ST):
                sc_ps = psQ.tile([P, T], f32, tag="qs", name=f"sc_{b}_{m}")
                i = 0
                for k in range(HT):
                    et = etp.tile([P, 2, 128], bf16, tag="et")
                    nc.sync.dma_start(
                        out=et,
                        in_=eT[b, 128 * k:128 * (k + 1), :,
                               128 * m:128 * (m + 1)])
                    for X, Y in ((0, 0), (0, 1), (1, 0)):
                        nc.tensor.matmul(sc_ps, et[:, X, :], qwhl[:, Y, k, :],
                                         start=(i == 0), stop=(i == 3 * HT - 1))
                        i += 1
                nc.scalar.copy(score[:, m, :], sc_ps)
                if m == 0:
                    nc.vector.tensor_copy(smax, sc_ps)
                else:
                    nc.vector.tensor_max(smax, smax, sc_ps)
            st[b] = (score, smax)

        def phase_B(b):
            # --- softmax: global max, exp (-> bf16 e), denominator ---
            score, smax = st[b]
            # max over masked-but-real rows is included; the uniform upward
            # shift cancels in the softmax ratio.
            smax_all = sp.tile([P, T], f32, tag="smax_all")
            nc.gpsimd.partition_all_reduce(smax_all, smax, channels=P,
                                           reduce_op=bass_isa.ReduceOp.max)
            e = pb.tile([P, ST, T], bf16, tag="big2")
            for m in range(ST):
                nc.vector.tensor_sub(score[:, m, :], score[:, m, :], smax_all)
                nc.scalar.activation(e[:, m, :], score[:, m, :], AF.Exp,
                                     bias=mask_sb[:, b, m:m + 1])
            # all-ones stationary -> den replicated on all 128 partitions
            den_ps = psD.tile([P, T], f32, tag="den", name=f"den_{b}")
            for m in range(ST):
                nc.tensor.matmul(den_ps, ones_sb, e[:, m, :],
                                 start=(m == 0), stop=(m == ST - 1))
            rdenb = sp.tile([P, T], f32, tag="rdenb")
            nc.vector.reciprocal(rdenb, den_ps)
            st[b] = (e, rdenb)

        def phase_C(b):
            # --- context (bf16) + output projection (fp16) ---
            e, rdenb = st[b]
            del st[b]
            enc_sb = enc_p.tile([P, ST, H], bf16, tag="enc")
            for k in range(ST):
                nc.sync.dma_start(out=enc_sb[:, k, :],
                                  in_=encb[b, 128 * k:128 * (k + 1), :])
            q16 = pq16.tile([P, HT, T], f16, tag="q16")
            nc.scalar.dma_start(
                out=q16,
                in_=q16_d[b, :, :].rearrange("(k p) t -> p k t", p=P))
            if b == 0:
                nc.sync.dma_start(
                    out=w16,
                    in_=W16[:, :, :].rearrange("(k p) c o -> p c k o", p=P))

            cn = pb.tile([P, HT, T], f16, tag="big1")
            for m in range(HT):
                c_ps = psC.tile([P, T], f32, tag="c", name=f"c_{b}_{m}")
                for k in range(ST):
                    nc.tensor.matmul(c_ps, enc_sb[:, k, 128 * m:128 * (m + 1)],
                                     e[:, k, :],
                                     start=(k == 0), stop=(k == ST - 1))
                nc.vector.tensor_mul(cn[:, m, :], c_ps, rdenb)

            for m in range(HT):
                o_ps = psO.tile([P, T], f32, tag="o", name=f"o_{b}_{m}")
                msl = slice(128 * m, 128 * (m + 1))
                for k in range(HT):
                    nc.tensor.matmul(o_ps, w16[:, 0, k, msl], q16[:, k, :],
                                     start=(k == 0), stop=False)
                for k in range(HT):
                    nc.tensor.matmul(o_ps, w16[:, 1, k, msl], cn[:, k, :],
                                     start=False, stop=(k == HT - 1))
                ot = otp.tile([P, T], f32, tag="ot")
                nc.scalar.activation(ot, o_ps, AF.Tanh, bias=bo_sb[:, m:m + 1])
                nc.scalar.dma_start(out=outT[b, 128 * m:128 * (m + 1), :],
                                    in_=ot)

        # software pipeline: A0 A1 B0 C0 B1 A2 C1 B2 A3 C2 B3 C3
        phase_A(0)
        phase_A(1)
        phase_B(0)
        phase_C(0)
        phase_B(1)
        phase_A(2)
        phase_C(1)
        phase_B(2)
        phase_A(3)
        phase_C(2)
        phase_B(3)
        phase_C(3)

    nc.compile()
    return nc


def _bf16_split(x):
    hi = x.astype(ml_dtypes.bfloat16)
    lo = (x - hi.astype(np.float32)).astype(ml_dtypes.bfloat16)
    return hi, lo


def kernel(query, encoder_outputs, src_lengths, W_in, W_out, b_out):
    query = np.asarray(query, dtype=np.float32)
    encoder_outputs = np.ascontiguousarray(np.asarray(encoder_outputs, np.float32))
    src_lengths = np.asarray(src_lengths)
    W_in = np.asarray(W_in, dtype=np.float32)
    W_out = np.asarray(W_out, dtype=np.float32)
    b_out = np.asarray(b_out, dtype=np.float32)

    # --- shared (weight) inputs ---
    W_inT = np.ascontiguousarray(W_in.T)                    # [h, o]
    Wih, Wil = _bf16_split(W_inT)
    W16 = np.ascontiguousarray(
        np.stack([W_out[:, :H].T, W_out[:, H:].T], axis=1)  # [h, 2, o]
    ).astype(np.float16)
    bo = np.ascontiguousarray(b_out.reshape(HT, P).T)       # [p, m]
    onesv = np.ones((P, P), dtype=ml_dtypes.bfloat16)

    # --- per-core shards ---
    in_maps = []
    for c in range(NCORES):
        bs = slice(c * BPC, (c + 1) * BPC)
        q = query[bs]                                       # [BPC, T, H]
        encs = encoder_outputs[bs]                          # [BPC, S, H]
        lens = np.asarray(src_lengths[bs], dtype=np.int64)

        qTa = np.ascontiguousarray(q.transpose(0, 2, 1))    # [BPC, H, T]
        qh, ql = _bf16_split(qTa)
        qhla = np.ascontiguousarray(np.stack([qh, ql], axis=2))  # [BPC, H, 2, T]
        q16a = qTa.astype(np.float16)                       # [BPC, H, T]
        encTa = np.ascontiguousarray(encs.transpose(0, 2, 1))  # [BPC, H, S]
        eh, el = _bf16_split(encTa)
        eTa = np.ascontiguousarray(np.stack([eh, el], axis=2))  # [BPC, H, 2, S]
        encba = encs.astype(ml_dtypes.bfloat16)             # [BPC, S, H]

        maskca = np.zeros((BPC, P, ST), dtype=np.float32)
        pos = (np.arange(ST)[None, :] * P + np.arange(P)[:, None])  # [P, ST]
        for j in range(BPC):
            maskca[j][pos >= lens[j]] = MASKVAL

        in_maps.append({
            "qhl": qhla, "q16": q16a, "eT": eTa, "encb": encba,
            "maskc": maskca, "Wih": Wih, "Wil": Wil, "W16": W16,
            "bo": bo, "onesv": onesv,
        })

    if not _nc_cache:
        _nc_cache.append(_build_nc())
    nc = _nc_cache[0]

    res = run_bass_kernel_spmd(nc, in_maps, core_ids=list(range(NCORES)),
                               trace=TRACE)
    global LAST_RESULTS
    LAST_RESULTS = res

    out = np.empty((B, T, H), dtype=np.float32)
    for c in range(NCORES):
        o = res.results[c]["outT"]                          # [BPC, H, T]
        out[c * BPC:(c + 1) * BPC] = o.transpose(0, 2, 1)
    return out


# revision 13
# speedup vs baseline: 1.3203x; 1.1184x over previous
"""Trainium2 Bass kernel for nn_Attention_12369505813001.

Computes, per batch b:
    qw    = query @ W_in.T                      [T, H]
    score = qw @ enc.T                          [T, S]
    p     = softmax(mask(score), axis=S)
    c     = p @ enc                             [T, H]
    out   = tanh(concat(query, c) @ W_out.T + b_out)

Shapes: B=32, T=512, S=1024, H=1024, fp32. Data-parallel over B across
8 NeuronCores (4 batches/core); no collectives.

Layout strategy (per core): keep the feature dim on partitions and T on
the free axis throughout ("transposed" layouts), so the PE contraction
dim always lands on partitions and no on-device transposes are needed:
    step1  qw^T[o,t]    = W_inT-tiles(stat) @ q^T(moving)     bf16x2, 3 MM/tile
    step2  score^T[s,t] = encT-tiles(stat)  @ qw^T(moving)    bf16x2
    softmax over s (partition+chunk axis): per-batch per-t max via
      free-axis reduce + GPSIMD partition all-reduce; exp on ACT with
      per-partition bias = additive length mask; denominator via an
      all-ones [128x128] stationary matmul (gives den broadcast to all
      128 partitions directly -> full-lane DVE reciprocal).
    step4  c~^T[h,t]    = enc-tiles(stat)   @ e^T(moving)     bf16
    step5  out^T[o,t]   = tanh(WqT(stat) @ q^T + WcT(stat) @ cnorm + b)  fp16

Precision: the softmax path (steps 1-2) uses two-term bf16 splits
(hi*hi + hi*lo + lo*hi accumulated in fp32 PSUM) — the softmax
amplifies score errors, so single-pass dtypes (bf16/f32r/fp16) all
fail the 2e-2 gate there. The context (step 4) runs in plain bf16 with
e stored bf16 (denominator computed from the same quantized e, so the
softmax simplex property is preserved), and step 5 runs in fp16
(11-bit mantissa at full PE rate, FWL-eligible unlike fp32r).
CPU-simulated end-to-end rel err of this mix: ~8.3e-3 (gate 2e-2).

Scheduling: the per-engine instruction streams are ordered at compile
time, so the batch loop is software-pipelined by hand into
A(load+step1+step2) / B(softmax+den) / C(context+step5) phases emitted
as A0 A1 B0 C0 B1 A2 C1 B2 A3 C2 B3 C3 — every B's vector/scalar
softmax work has another batch's matmuls in flight on the PE.
SBUF big tiles rotate through bufs=2 pool tags
(big1: score->cn16, big2: qwhl->e); this emission order keeps every
tag's producer after the previous consumer. DMAs are spread across the
two HWDGE queues (sync: weights/encT/enc; scalar: q-side + stores).
"""

from contextlib import ExitStack

import numpy as np
import ml_dtypes

import concourse.bass as bass
import concourse.bass_isa as bass_isa
import concourse.mybir as mybir
import concourse.tile as tile
from concourse import bacc
from concourse.bass_utils import run_bass_kernel_spmd

B, T, S, H = 32, 512, 1024, 1024
NCORES = 8
BPC = B // NCORES          # batches per core
HT = H // 128              # h/o chunk count
ST = S // 128              # s chunk count
P = 128

f32 = mybir.dt.float32
bf16 = mybir.dt.bfloat16
f16 = mybir.dt.float16
AX = mybir.AxisListType.X
AF = mybir.ActivationFunctionType

MASKVAL = -1.0e38

_nc_cache = []

TRACE = False          # set by test.py to capture an NTFF/perfetto profile
LAST_RESULTS = None    # test.py reads exec_time_ns / trace path from here


def _build_nc():
    nc = bacc.Bacc("TRN2", target_bir_lowering=False, debug=False)

    qhl_d = nc.dram_tensor("qhl", [BPC, H, 2, T], bf16, kind="ExternalInput")
    q16_d = nc.dram_tensor("q16", [BPC, H, T], f16, kind="ExternalInput")
    # eT pre-tiled on host: [b, k, p, m2, hi/lo, 256] so each per-partition
    # DMA line is 1KB contiguous (2x128-col score tiles per load)
    eT = nc.dram_tensor("eT", [BPC, HT, P, ST // 2, 2, 256], bf16,
                        kind="ExternalInput")
    encb = nc.dram_tensor("encb", [BPC, S, H], bf16, kind="ExternalInput")
    maskc = nc.dram_tensor("maskc", [BPC, P, ST], f32, kind="ExternalInput")
    Wih = nc.dram_tensor("Wih", [H, H], bf16, kind="ExternalInput")
    Wil = nc.dram_tensor("Wil", [H, H], bf16, kind="ExternalInput")
    W16 = nc.dram_tensor("W16", [H, 2, H], f16, kind="ExternalInput")
    bo = nc.dram_tensor("bo", [P, HT], f32, kind="ExternalInput")
    onesv = nc.dram_tensor("onesv", [P, P], bf16, kind="ExternalInput")
    outT = nc.dram_tensor("outT", [BPC, H, T], f32, kind="ExternalOutput")

    with tile.TileContext(nc) as tc, ExitStack() as ctx:
        wp = ctx.enter_context(tc.tile_pool(name="wp", bufs=1))
        pq = ctx.enter_context(tc.tile_pool(name="pq", bufs=2))
        pq16 = ctx.enter_context(tc.tile_pool(name="pq16", bufs=1))
        pb = ctx.enter_context(tc.tile_pool(name="pb", bufs=2))
        sp = ctx.enter_context(tc.tile_pool(name="sp", bufs=2))
        sp1 = ctx.enter_context(tc.tile_pool(name="sp1", bufs=1))
        etp = ctx.enter_context(tc.tile_pool(name="etp", bufs=10))
        enc_p = ctx.enter_context(tc.tile_pool(name="enc_p", bufs=1))
        otp = ctx.enter_context(tc.tile_pool(name="otp", bufs=2))
        psQ = ctx.enter_context(tc.tile_pool(name="psQ", bufs=2, space="PSUM"))
        psO = ctx.enter_context(tc.tile_pool(name="psO", bufs=2, space="PSUM"))
        psC = ctx.enter_context(tc.tile_pool(name="psC", bufs=3, space="PSUM"))
        psD = ctx.enter_context(tc.tile_pool(name="psD", bufs=1, space="PSUM"))

        # --- persistent weights, interleaved per-k with batch 0's q load so
        # the first step-1 matmul starts after ~1/8 of the weight traffic
        # (w16 is emitted later: first use is C0) ---
        wih = wp.tile([P, HT, H], bf16)
        wil = wp.tile([P, HT, H], bf16)
        qhl0 = pq.tile([P, 2, HT, T], bf16, tag="qhl")
        for k in range(HT):
            ksl = slice(128 * k, 128 * (k + 1))
            nc.sync.dma_start(out=wih[:, k, :], in_=Wih[ksl, :])
            nc.scalar.dma_start(out=qhl0[:, :, k, :], in_=qhl_d[0, ksl, :, :])
            nc.sync.dma_start(out=wil[:, k, :], in_=Wil[ksl, :])
        bo_sb = wp.tile([P, HT], f32)
        nc.sync.dma_start(out=bo_sb, in_=bo[:, :])
        mask_sb = wp.tile([P, BPC, ST], f32)
        nc.sync.dma_start(out=mask_sb, in_=maskc[:, :, :].rearrange("b p m -> p b m"))
        ones_sb = wp.tile([P, P], bf16)
        nc.sync.dma_start(out=ones_sb, in_=onesv[:, :])
        w16 = wp.tile([P, 2, HT, H], f16)

        st = {}

        def phase_A(b):
            # --- load q^T hi/lo (host-split); step 1; step 2 + max tree ---
            if b == 0:
                qhl = qhl0
            else:
                qhl = pq.tile([P, 2, HT, T], bf16, tag="qhl")
                for k in range(HT):
                    nc.scalar.dma_start(
                        out=qhl[:, :, k, :],
                        in_=qhl_d[b, 128 * k:128 * (k + 1), :, :])

            qwhl = pb.tile([P, 2, HT, T], bf16, tag="big2")
            for m in range(HT):
                qw_ps = psQ.tile([P, T], f32, tag="qs", name=f"qw_{b}_{m}")
                i = 0
                msl = slice(128 * m, 128 * (m + 1))
                for k in range(HT):
                    for X, Y in ((wih, 0), (wih, 1), (wil, 0)):
                        nc.tensor.matmul(qw_ps, X[:, k, msl], qhl[:, Y, k, :],
                                         start=(i == 0), stop=(i == 3 * HT - 1))
                        i += 1
                nc.scalar.copy(qwhl[:, 0, m, :], qw_ps)
                nc.vector.tensor_sub(qwhl[:, 1, m, :], qw_ps, qwhl[:, 0, m, :])

            score = pb.tile([P, ST, T], f32, tag="big1")
            smax = sp.tile([P, T], f32, tag="smax")
            for m2 in range(ST // 2):
                ets = []
                for k in range(HT):
                    et = etp.tile([P, 2, 256], bf16, tag="et")
                    nc.sync.dma_start(out=et, in_=eT[b, k, :, m2, :, :])
                    ets.append(et)
                for j in range(2):
                    m = 2 * m2 + j
                    jsl = slice(128 * j, 128 * (j + 1))
                    sc_ps = psQ.tile([P, T], f32, tag="qs", name=f"sc_{b}_{m}")
                    i = 0
                    for k in range(HT):
                        for X, Y in ((0, 0), (0, 1), (1, 0)):
                            nc.tensor.matmul(sc_ps, ets[k][:, X, jsl],
                                             qwhl[:, Y, k, :],
                                             start=(i == 0),
                                             stop=(i == 3 * HT - 1))
                            i += 1
                    nc.scalar.copy(score[:, m, :], sc_ps)
                    if m == 0:
                        nc.vector.tensor_copy(smax, sc_ps)
                    else:
                        nc.vector.tensor_max(smax, smax, sc_ps)
            st[b] = (score, smax)

        def phase_B(b):
            # --- softmax: global max, exp (-> bf16 e), denominator ---
            score, smax = st[b]
            # max over masked-but-real rows is included; the uniform upward
            # shift cancels in the softmax ratio.
            smax_all = sp1.tile([P, T], f32, tag="smax_all")
            nc.gpsimd.partition_all_reduce(smax_all, smax, channels=P,
                                           reduce_op=bass_isa.ReduceOp.max)
            e = pb.tile([P, ST, T], bf16, tag="big2")
            for m in range(ST):
                nc.vector.tensor_sub(score[:, m, :], score[:, m, :], smax_all)
                nc.scalar.activation(e[:, m, :], score[:, m, :], AF.Exp,
                                     bias=mask_sb[:, b, m:m + 1])
            # all-ones stationary -> den replicated on all 128 partitions
            den_ps = psD.tile([P, T], f32, tag="den", name=f"den_{b}")
            for m in range(ST):
                nc.tensor.matmul(den_ps, ones_sb, e[:, m, :],
                                 start=(m == 0), stop=(m == ST - 1))
            rdenb = sp1.tile([P, T], f32, tag="rdenb")
            nc.vector.reciprocal(rdenb, den_ps)
            st[b] = (e, rdenb)

        def phase_C(b):
            # --- context (bf16) + output projection (fp16) ---
            e, rdenb = st[b]
            del st[b]
            enc_sb = enc_p.tile([P, ST, H], bf16, tag="enc")
            for k in range(ST):
                nc.sync.dma_start(out=enc_sb[:, k, :],
                                  in_=encb[b, 128 * k:128 * (k + 1), :])
            q16 = pq16.tile([P, HT, T], f16, tag="q16")
            nc.scalar.dma_start(
                out=q16,
                in_=q16_d[b, :, :].rearrange("(k p) t -> p k t", p=P))
            if b == 0:
                nc.sync.dma_start(
                    out=w16,
                    in_=W16[:, :, :].rearrange("(k p) c o -> p c k o", p=P))

            cn = pb.tile([P, HT, T], f16, tag="big1")
            for m in range(HT):
                c_ps = psC.tile([P, T], f32, tag="c", name=f"c_{b}_{m}")
                for k in range(ST):
                    nc.tensor.matmul(c_ps, enc_sb[:, k, 128 * m:128 * (m + 1)],
                                     e[:, k, :],
                                     start=(k == 0), stop=(k == ST - 1))
                nc.vector.tensor_mul(cn[:, m, :], c_ps, rdenb)

            for m in range(HT):
                o_ps = psO.tile([P, T], f32, tag="o", name=f"o_{b}_{m}")
                msl = slice(128 * m, 128 * (m + 1))
                for k in range(HT):
                    nc.tensor.matmul(o_ps, w16[:, 0, k, msl], q16[:, k, :],
                                     start=(k == 0), stop=False)
                for k in range(HT):
                    nc.tensor.matmul(o_ps, w16[:, 1, k, msl], cn[:, k, :],
                                     start=False, stop=(k == HT - 1))
                ot = otp.tile([P, T], f32, tag="ot")
                nc.scalar.activation(ot, o_ps, AF.Tanh, bias=bo_sb[:, m:m + 1])
                nc.sync.dma_start(out=outT[b, 128 * m:128 * (m + 1), :],
                                  in_=ot)

        # software pipeline: A0 A1 B0 C0 B1 A2 C1 B2 A3 C2 B3 C3
        phase_A(0)
        phase_A(1)
        phase_B(0)
        phase_C(0)
        phase_B(1)
        phase_A(2)
        phase_C(1)
        phase_B(2)
        phase_A(3)
        phase_C(2)
        phase_B(3)
        phase_C(3)

    nc.compile()
    return nc


def _bf16_split(x):
    hi = x.astype(ml_dtypes.bfloat16)
    lo = (x - hi.astype(np.float32)).astype(ml_dtypes.bfloat16)
    return hi, lo


def kernel(query, encoder_outputs, src_lengths, W_in, W_out, b_out):
    query = np.asarray(query, dtype=np.float32)
    encoder_outputs = np.ascontiguousarray(np.asarray(encoder_outputs, np.float32))
    src_lengths = np.asarray(src_lengths)
    W_in = np.asarray(W_in, dtype=np.float32)
    W_out = np.asarray(W_out, dtype=np.float32)
    b_out = np.asarray(b_out, dtype=np.float32)

    # --- shared (weight) inputs ---
    W_inT = np.ascontiguousarray(W_in.T)                    # [h, o]
    Wih, Wil = _bf16_split(W_inT)
    W16 = np.ascontiguousarray(
        np.stack([W_out[:, :H].T, W_out[:, H:].T], axis=1)  # [h, 2, o]
    ).astype(np.float16)
    bo = np.ascontiguousarray(b_out.reshape(HT, P).T)       # [p, m]
    onesv = np.ones((P, P), dtype=ml_dtypes.bfloat16)

    # --- per-core shards ---
    in_maps = []
    for c in range(NCORES):
        bs = slice(c * BPC, (c + 1) * BPC)
        q = query[bs]                                       # [BPC, T, H]
        encs = encoder_outputs[bs]                          # [BPC, S, H]
        lens = np.asarray(src_lengths[bs], dtype=np.int64)

        qTa = np.ascontiguousarray(q.transpose(0, 2, 1))    # [BPC, H, T]
        qh, ql = _bf16_split(qTa)
        qhla = np.ascontiguousarray(np.stack([qh, ql], axis=2))  # [BPC, H, 2, T]
        q16a = qTa.astype(np.float16)                       # [BPC, H, T]
        encTa = np.ascontiguousarray(encs.transpose(0, 2, 1))  # [BPC, H, S]
        eh, el = _bf16_split(encTa)
        # pre-tile for 1KB DMA lines: [b, k, p, m2, hi/lo, 256]
        ehr = eh.reshape(BPC, HT, P, ST // 2, 256)
        elr = el.reshape(BPC, HT, P, ST // 2, 256)
        eTa = np.ascontiguousarray(np.stack([ehr, elr], axis=4))
        encba = encs.astype(ml_dtypes.bfloat16)             # [BPC, S, H]

        maskca = np.zeros((BPC, P, ST), dtype=np.float32)
        pos = (np.arange(ST)[None, :] * P + np.arange(P)[:, None])  # [P, ST]
        for j in range(BPC):
            maskca[j][pos >= lens[j]] = MASKVAL

        in_maps.append({
            "qhl": qhla, "q16": q16a, "eT": eTa, "encb": encba,
            "maskc": maskca, "Wih": Wih, "Wil": Wil, "W16": W16,
            "bo": bo, "onesv": onesv,
        })

    if not _nc_cache:
        _nc_cache.append(_build_nc())
    nc = _nc_cache[0]

    res = run_bass_kernel_spmd(nc, in_maps, core_ids=list(range(NCORES)),
                               trace=TRACE)
    global LAST_RESULTS
    LAST_RESULTS = res

    out = np.empty((B, T, H), dtype=np.float32)
    for c in range(NCORES):
        o = res.results[c]["outT"]                          # [BPC, H, T]
        out[c * BPC:(c + 1) * BPC] = o.transpose(0, 2, 1)
    return out


# revision 14
# speedup vs baseline: 1.5083x; 1.1424x over previous
"""Trainium2 Bass kernel for nn_Attention_12369505813001.

Computes, per batch b:
    qw    = query @ W_in.T                      [T, H]
    score = qw @ enc.T                          [T, S]
    p     = softmax(mask(score), axis=S)
    c     = p @ enc                             [T, H]
    out   = tanh(concat(query, c) @ W_out.T + b_out)

Shapes: B=32, T=512, S=1024, H=1024, fp32. Data-parallel over B across
8 NeuronCores (4 batches/core); no collectives.

Layout strategy (per core): keep the feature dim on partitions and T on
the free axis throughout ("transposed" layouts), so the PE contraction
dim always lands on partitions and no on-device transposes are needed.

Precision: the softmax path needs ~14+ effective mantissa bits (the
softmax exponentiates score errors; score std is sqrt(H)=32), which no
single-pass PE dtype provides. Steps 1-2 therefore run a two-pass
scheme per 128x128 tile at ~16 effective bits:
    hi@hi   in fp16 (11-bit mantissa, 1 row/cycle), moving operand
            pre-scaled by 2^12
    cross   hi@lo + lo@hi packed into ONE fp8-e4m3 DoubleRow matmul
            (two virtual k-subtiles, 0.5 rows/cycle); per-slot scale
            factors are chosen so both products also come out at 2^12
Both accumulate into the SAME fp32 PSUM bank; one Copy-activation with
scale=2^-12 recovers the fp32 result. The correction terms are 2^-12
of the main term, so fp8's 4-bit relative accuracy on them lands at
~2^-16. The context (step 4) runs in plain bf16 with e stored bf16
(denominator computed from the same quantized e preserves the softmax
simplex), and step 5 runs in fp16. CPU-simulated end-to-end rel err of
this exact scheme: 8.4e-3 (1.1e-2 if the PE flushes fp16 subnormals);
gate is 2e-2.

Scheduling: per-engine instruction streams are ordered at compile
time, so the batch loop is software-pipelined by hand into
A(load+step1+step2) / B(softmax+den) / C(context+step5) phases emitted
as A0 A1 B0 C0 B1 A2 C1 B2 A3 C2 B3 C3 — every B's vector/scalar
softmax work has another batch's matmuls in flight on the PE.
The denominator uses an all-ones [128x128] stationary matmul so den
lands broadcast on all 128 partitions (full-lane DVE reciprocal, no
partition ops). DMAs are spread across both HWDGE queues with 1-2KB
per-partition lines (host pre-tiles eT accordingly).
"""

from contextlib import ExitStack

import numpy as np
import ml_dtypes

import concourse.bass as bass
import concourse.bass_isa as bass_isa
import concourse.mybir as mybir
import concourse.tile as tile
from concourse import bacc
from concourse.bass_utils import run_bass_kernel_spmd

B, T, S, H = 32, 512, 1024, 1024
NCORES = 8
BPC = B // NCORES          # batches per core
HT = H // 128              # h/o chunk count
ST = S // 128              # s chunk count
P = 128

f32 = mybir.dt.float32
bf16 = mybir.dt.bfloat16
f16 = mybir.dt.float16
fp8 = mybir.dt.float8e4
AX = mybir.AxisListType.X
AF = mybir.ActivationFunctionType
ALU = mybir.AluOpType
DR = mybir.MatmulPerfMode.DoubleRow

MASKVAL = -1.0e38
SC = 2.0 ** 12             # shared product scale of the split matmuls

_nc_cache = []

TRACE = False          # set by test.py to capture an NTFF/perfetto profile
LAST_RESULTS = None    # test.py reads exec_time_ns / trace path from here


def _build_nc():
    nc = bacc.Bacc("TRN2", target_bir_lowering=False, debug=False)

    q16s_d = nc.dram_tensor("q16s", [BPC, H, T], f16, kind="ExternalInput")
    q8_d = nc.dram_tensor("q8", [BPC, H, 2, T], fp8, kind="ExternalInput")
    q16_d = nc.dram_tensor("q16", [BPC, H, T], f16, kind="ExternalInput")
    # eT pre-tiled on host: [b, k, p, m2, (slots,) 256] so per-partition DMA
    # lines are contiguous (2x128-col score tiles per load)
    eT16 = nc.dram_tensor("eT16", [BPC, HT, P, ST // 2, 256], f16,
                          kind="ExternalInput")
    eT8 = nc.dram_tensor("eT8", [BPC, HT, P, ST // 2, 2, 256], fp8,
                         kind="ExternalInput")
    encb = nc.dram_tensor("encb", [BPC, S, H], bf16, kind="ExternalInput")
    maskc = nc.dram_tensor("maskc", [BPC, P, ST], f32, kind="ExternalInput")
    W1h_d = nc.dram_tensor("W1h", [H, H], f16, kind="ExternalInput")
    W1c_d = nc.dram_tensor("W1c", [H, 2, H], fp8, kind="ExternalInput")
    W16 = nc.dram_tensor("W16", [H, 2, H], f16, kind="ExternalInput")
    bo = nc.dram_tensor("bo", [P, HT], f32, kind="ExternalInput")
    onesv = nc.dram_tensor("onesv", [P, P], bf16, kind="ExternalInput")
    outT = nc.dram_tensor("outT", [BPC, H, T], f32, kind="ExternalOutput")

    with tile.TileContext(nc) as tc, ExitStack() as ctx:
        wp = ctx.enter_context(tc.tile_pool(name="wp", bufs=1))
        pq = ctx.enter_context(tc.tile_pool(name="pq", bufs=2))
        pq16 = ctx.enter_context(tc.tile_pool(name="pq16", bufs=1))
        pb = ctx.enter_context(tc.tile_pool(name="pb", bufs=2))
        sp = ctx.enter_context(tc.tile_pool(name="sp", bufs=2))
        sp1 = ctx.enter_context(tc.tile_pool(name="sp1", bufs=1))
        etp = ctx.enter_context(tc.tile_pool(name="etp", bufs=10))
        enc_p = ctx.enter_context(tc.tile_pool(name="enc_p", bufs=1))
        otp = ctx.enter_context(tc.tile_pool(name="otp", bufs=2))
        psQ = ctx.enter_context(tc.tile_pool(name="psQ", bufs=2, space="PSUM"))
        psO = ctx.enter_context(tc.tile_pool(name="psO", bufs=2, space="PSUM"))
        psC = ctx.enter_context(tc.tile_pool(name="psC", bufs=3, space="PSUM"))
        psD = ctx.enter_context(tc.tile_pool(name="psD", bufs=1, space="PSUM"))

        # --- persistent weights, interleaved per-k with batch 0's q load so
        # the first step-1 matmul starts after ~1/8 of the weight traffic
        # (w16 is emitted later: first use is C0) ---
        w1h = wp.tile([P, HT, H], f16)
        w1c = wp.tile([P, HT, 2, H], fp8)
        q16s0 = pq.tile([P, HT, T], f16, tag="q16s")
        q8s0 = pq.tile([P, HT, 2, T], fp8, tag="q8")
        for k in range(HT):
            ksl = slice(128 * k, 128 * (k + 1))
            nc.sync.dma_start(out=w1h[:, k, :], in_=W1h_d[ksl, :])
            nc.scalar.dma_start(out=q16s0[:, k, :], in_=q16s_d[0, ksl, :])
            nc.sync.dma_start(out=w1c[:, k, :, :], in_=W1c_d[ksl, :, :])
            nc.scalar.dma_start(out=q8s0[:, k, :, :], in_=q8_d[0, ksl, :, :])
        bo_sb = wp.tile([P, HT], f32)
        nc.sync.dma_start(out=bo_sb, in_=bo[:, :])
        mask_sb = wp.tile([P, BPC, ST], f32)
        nc.sync.dma_start(out=mask_sb, in_=maskc[:, :, :].rearrange("b p m -> p b m"))
        ones_sb = wp.tile([P, P], bf16)
        nc.sync.dma_start(out=ones_sb, in_=onesv[:, :])
        w16 = wp.tile([P, 2, HT, H], f16)

        st = {}

        def phase_A(b):
            # --- load q side; step 1; step 2 + max tree ---
            if b == 0:
                q16s, q8s = q16s0, q8s0
            else:
                q16s = pq.tile([P, HT, T], f16, tag="q16s")
                q8s = pq.tile([P, HT, 2, T], fp8, tag="q8")
                for k in range(HT):
                    ksl = slice(128 * k, 128 * (k + 1))
                    nc.scalar.dma_start(out=q16s[:, k, :], in_=q16s_d[b, ksl, :])
                    nc.scalar.dma_start(out=q8s[:, k, :, :], in_=q8_d[b, ksl, :, :])

            # step 1: PSUM accumulates 2^12 * qw
            qwh = pb.tile([P, HT, T], f16, tag="big2a")      # fp16(2^12 qw)
            qw8c = pb.tile([P, HT, 2, T], fp8, tag="big2b")  # [4096*qwlo, qw]
            for m in range(HT):
                qw_ps = psQ.tile([P, T], f32, tag="qs", name=f"qw_{b}_{m}")
                msl = slice(128 * m, 128 * (m + 1))
                for k in range(HT):
                    nc.tensor.matmul(qw_ps, w1h[:, k, msl], q16s[:, k, :],
                                     start=(k == 0), stop=False)
                    nc.tensor.matmul(qw_ps, w1c[:, k, :, msl], q8s[:, k, :, :],
                                     perf_mode=DR,
                                     start=False, stop=(k == HT - 1))
                nc.scalar.copy(qwh[:, m, :], qw_ps)
                nc.scalar.activation(qw8c[:, m, 1, :], qw_ps, AF.Copy,
                                     scale=1.0 / SC)
                nc.vector.tensor_sub(qw8c[:, m, 0, :], qw_ps, qwh[:, m, :])

            # step 2: PSUM accumulates 2^12 * score
            score = pb.tile([P, ST, T], f32, tag="big1")
            smax = sp.tile([P, T], f32, tag="smax")
            for m2 in range(ST // 2):
                ets16, ets8 = [], []
                for k in range(HT):
                    et16 = etp.tile([P, 256], f16, tag="et16")
                    nc.sync.dma_start(out=et16, in_=eT16[b, k, :, m2, :])
                    et8 = etp.tile([P, 2, 256], fp8, tag="et8")
                    nc.sync.dma_start(out=et8, in_=eT8[b, k, :, m2, :, :])
                    ets16.append(et16)
                    ets8.append(et8)
                for j in range(2):
                    m = 2 * m2 + j
                    jsl = slice(128 * j, 128 * (j + 1))
                    sc_ps = psQ.tile([P, T], f32, tag="qs", name=f"sc_{b}_{m}")
                    for k in range(HT):
                        nc.tensor.matmul(sc_ps, ets16[k][:, jsl], qwh[:, k, :],
                                         start=(k == 0), stop=False)
                        nc.tensor.matmul(sc_ps, ets8[k][:, :, jsl],
                                         qw8c[:, k, :, :], perf_mode=DR,
                                         start=False, stop=(k == HT - 1))
                    nc.scalar.activation(score[:, m, :], sc_ps, AF.Copy,
                                         scale=1.0 / SC)
                    if m == 0:
                        nc.vector.tensor_copy(smax, score[:, m, :])
                    else:
                        nc.vector.tensor_max(smax, smax, score[:, m, :])
            st[b] = (score, smax)

        def phase_B(b):
            # --- softmax: global max, exp (-> bf16 e), denominator ---
            score, smax = st[b]
            # max over masked-but-real rows is included; the uniform upward
            # shift cancels in the softmax ratio.
            smax_all = sp1.tile([P, T], f32, tag="smax_all")
            nc.gpsimd.partition_all_reduce(smax_all, smax, channels=P,
                                           reduce_op=bass_isa.ReduceOp.max)
            e = pb.tile([P, ST, T], bf16, tag="big2a")
            for m in range(ST):
                nc.vector.tensor_sub(score[:, m, :], score[:, m, :], smax_all)
                nc.scalar.activation(e[:, m, :], score[:, m, :], AF.Exp,
                                     bias=mask_sb[:, b, m:m + 1])
            # all-ones stationary -> den replicated on all 128 partitions
            den_ps = psD.tile([P, T], f32, tag="den", name=f"den_{b}")
            for m in range(ST):
                nc.tensor.matmul(den_ps, ones_sb, e[:, m, :],
                                 start=(m == 0), stop=(m == ST - 1))
            rdenb = sp1.tile([P, T], f32, tag="rdenb")
            nc.vector.reciprocal(rdenb, den_ps)
            st[b] = (e, rdenb)

        def phase_C(b):
            # --- context (bf16) + output projection (fp16) ---
            e, rdenb = st[b]
            del st[b]
            enc_sb = enc_p.tile([P, ST, H], bf16, tag="enc")
            for k in range(ST):
                nc.sync.dma_start(out=enc_sb[:, k, :],
                                  in_=encb[b, 128 * k:128 * (k + 1), :])
            q16 = pq16.tile([P, HT, T], f16, tag="q16")
            nc.scalar.dma_start(
                out=q16,
                in_=q16_d[b, :, :].rearrange("(k p) t -> p k t", p=P))
            if b == 0:
                nc.sync.dma_start(
                    out=w16,
                    in_=W16[:, :, :].rearrange("(k p) c o -> p c k o", p=P))

            cn = pb.tile([P, HT, T], f16, tag="big1")
            for m in range(HT):
                c_ps = psC.tile([P, T], f32, tag="c", name=f"c_{b}_{m}")
                for k in range(ST):
                    nc.tensor.matmul(c_ps, enc_sb[:, k, 128 * m:128 * (m + 1)],
                                     e[:, k, :],
                                     start=(k == 0), stop=(k == ST - 1))
                nc.vector.tensor_mul(cn[:, m, :], c_ps, rdenb)

            for m in range(HT):
                o_ps = psO.tile([P, T], f32, tag="o", name=f"o_{b}_{m}")
                msl = slice(128 * m, 128 * (m + 1))
                for k in range(HT):
                    nc.tensor.matmul(o_ps, w16[:, 0, k, msl], q16[:, k, :],
                                     start=(k == 0), stop=False)
                for k in range(HT):
                    nc.tensor.matmul(o_ps, w16[:, 1, k, msl], cn[:, k, :],
                                     start=False, stop=(k == HT - 1))
                ot = otp.tile([P, T], f32, tag="ot")
                nc.scalar.activation(ot, o_ps, AF.Tanh, bias=bo_sb[:, m:m + 1])
                nc.sync.dma_start(out=outT[b, 128 * m:128 * (m + 1), :],
                                  in_=ot)

        # software pipeline: A0 A1 B0 C0 B1 A2 C1 B2 A3 C2 B3 C3
        phase_A(0)
        phase_A(1)
        phase_B(0)
        phase_C(0)
        phase_B(1)
        phase_A(2)
        phase_C(1)
        phase_B(2)
        phase_A(3)
        phase_C(2)
        phase_B(3)
        phase_C(3)

    nc.compile()
    return nc


def _f16_split(x):
    hi = x.astype(np.float16).astype(np.float32)
    return hi, x - hi


def _f8(x):
    return x.astype(ml_dtypes.float8_e4m3)


def kernel(query, encoder_outputs, src_lengths, W_in, W_out, b_out):
    query = np.asarray(query, dtype=np.float32)
    encoder_outputs = np.ascontiguousarray(np.asarray(encoder_outputs, np.float32))
    src_lengths = np.asarray(src_lengths)
    W_in = np.asarray(W_in, dtype=np.float32)
    W_out = np.asarray(W_out, dtype=np.float32)
    b_out = np.asarray(b_out, dtype=np.float32)

    # --- shared (weight) inputs ---
    W_inT = np.ascontiguousarray(W_in.T)                    # [h, o]
    _, Wlo = _f16_split(W_inT)
    W1h = W_inT.astype(np.float16)
    W1c = np.ascontiguousarray(
        np.stack([_f8(8.0 * W_inT), _f8(16384.0 * Wlo)], axis=1))  # [h, 2, o]
    W16 = np.ascontiguousarray(
        np.stack([W_out[:, :H].T, W_out[:, H:].T], axis=1)  # [h, 2, o]
    ).astype(np.float16)
    bo = np.ascontiguousarray(b_out.reshape(HT, P).T)       # [p, m]
    onesv = np.ones((P, P), dtype=ml_dtypes.bfloat16)

    # --- per-core shards ---
    in_maps = []
    for c in range(NCORES):
        bs = slice(c * BPC, (c + 1) * BPC)
        q = query[bs]                                       # [BPC, T, H]
        encs = encoder_outputs[bs]                          # [BPC, S, H]
        lens = np.asarray(src_lengths[bs], dtype=np.int64)

        qTa = np.ascontiguousarray(q.transpose(0, 2, 1))    # [BPC, H, T]
        _, qlo = _f16_split(qTa)
        q16sa = (4096.0 * qTa).astype(np.float16)           # fp16(2^12 q)
        q8a = np.ascontiguousarray(
            np.stack([_f8(512.0 * qlo), _f8(0.25 * qTa)], axis=2))  # [b,h,2,t]
        q16a = qTa.astype(np.float16)                       # [BPC, H, T]
        encTa = np.ascontiguousarray(encs.transpose(0, 2, 1))  # [BPC, H, S]
        eh, elo = _f16_split(encTa)
        # pre-tile for contiguous DMA lines: [b, k, p, m2, (slots,) 256]
        eT16a = np.ascontiguousarray(
            eh.astype(np.float16).reshape(BPC, HT, P, ST // 2, 256))
        e8hi = _f8(encTa).reshape(BPC, HT, P, ST // 2, 256)
        e8lo = _f8(4096.0 * elo).reshape(BPC, HT, P, ST // 2, 256)
        eT8a = np.ascontiguousarray(np.stack([e8hi, e8lo], axis=4))
        encba = encs.astype(ml_dtypes.bfloat16)             # [BPC, S, H]

        maskca = np.zeros((BPC, P, ST), dtype=np.float32)
        pos = (np.arange(ST)[None, :] * P + np.arange(P)[:, None])  # [P, ST]
        for j in range(BPC):
            maskca[j][pos >= lens[j]] = MASKVAL

        in_maps.append({
            "q16s": q16sa, "q8": q8a, "q16": q16a,
            "eT16": eT16a, "eT8": eT8a, "encb": encba,
            "maskc": maskca, "W1h": W1h, "W1c": W1c, "W16": W16,
            "bo": bo, "onesv": onesv,
        })

    if not _nc_cache:
        _nc_cache.append(_build_nc())
    nc = _nc_cache[0]

    res = run_bass_kernel_spmd(nc, in_maps, core_ids=list(range(NCORES)),
                               trace=TRACE)
    global LAST_RESULTS
    LAST_RESULTS = res

    out = np.empty((B, T, H), dtype=np.float32)
    for c in range(NCORES):
        o = res.results[c]["outT"]                          # [BPC, H, T]
        out[c * BPC:(c + 1) * BPC] = o.transpose(0, 2, 1)
    return out
